# revision 33
# baseline (speedup 1.0000x reference)
"""Trainium2 Bass kernel for nn_EuclideanNet (gnn_message_passing).

Math: for each sample z, with points g[b] in R^3 and features f[b] in R^23:
    r_ab   = sqrt(max(|g_a - g_b|^2, 1e-12))
    K(r)   = Y00 * (relu(basis(r) @ W1 + b1) @ W2 + b2)      (23-vector, fn of r only)
    conv_a = sum_b <K(r_ab), f_b> / sqrt(N)
    out_z  = relu-MLP head (512 -> 30 -> 10 -> 1) on conv

K(r) is a fixed scalar->R^23 map, zero for r >= 4.5.  With
phi = min(r,4.5)*pi/4.5 we fit  K_c(r) ~= sum_q alpha[q,c] T_q(phi)  where
T_q are {1, phi, tanh(s(phi-c)), relu(phi-c), relu(phi-c)^2} columns chosen
by a greedy forward selection at runtime.  The whole conv then becomes
PSUM-accumulated rank-1 matmuls over fp16 pair tiles:
    conv[a] = sum_q sum_b  g[q,b] * T_q(phi[b,a]),
    g[q,b]  = sum_c alpha[q,c] f[b,c]/sqrt(N)

Perf structure (per core; 2 samples, 512x512 pairs each):
 - everything pairwise is fp16: DVE tensor_scalar hits the 4x perf mode
   (~1.2us per [128,4096] hinge column) and the PE streams 1 col/cycle.
 - conv matmuls use 128x32 column tiling: 4 concurrent PE tiles, one per
   emission lane (tile_position=(0,32*lane)), each accumulating a partial
   conv row in its own PSUM partition quadrant; rows summed at the end.
 - producer columns are split across DVE (hinges), ACT (tanh), and GPSIMD
   (hinges) so all engines run concurrently; emission order merges the
   per-engine queues by simulated finish time.
 - r^2 pair matmuls run in fp32r (1 cycle/row vs 4 for fp32).

Sharding: pure data parallel, 2 samples per core across 8 cores.
"""

import math
import os

import numpy as np

import concourse.bass as bass
import concourse.bacc as bacc
import concourse.mybir as mybir
import concourse.tile as tile
from contextlib import ExitStack

# ----------------------------------------------------------------------------
# problem constants (hardcoded per the harness contract)
B = 16
N = 512
C = 23
NCORES = 8
BPER = B // NCORES          # samples per core
MAX_RADIUS = 3.0
N_BASIS = 3
RCUT = 4.5                  # K(r) == 0 for r >= RCUT
Y00 = 1.0 / (2.0 * math.sqrt(math.pi))
NPAIR = BPER * 4 * N        # free extent of the (z, bchunk, a) pair layout

# candidate tanh knots (s, c): sigmoid dictionary covering phi in [0, pi]
CAND_TANH = [
    (0.641464, -0.565098),
    (5.199581, 0.083066),
    (11.662816, 0.228894),
    (5.808065, 0.494841),
    (4.751950, 0.688576),
    (5.791623, 0.905913),
    (6.042989, 1.035341),
    (2.708561, 1.102941),
    (5.956410, 1.155777),
    (7.073416, 1.278710),
    (1.013782, 1.281423),
    (0.735399, 1.581629),
    (18.198025, 1.608823),
    (3.140800, 1.640201),
    (9.748936, 1.938329),
    (10.659949, 1.945228),
    (5.877438, 2.067339),
    (3.977013, 2.228743),
    (4.346255, 2.415862),
    (4.060595, 2.596987),
    (3.609979, 3.034559),
    (3.984574, 3.318295),
    (0.103047, 4.977751),
]

F32 = mybir.dt.float32
F32R = mybir.dt.float32r
F16 = mybir.dt.float16
AF = mybir.ActivationFunctionType
ALU = mybir.AluOpType

# producer op cost (us) per [128, NPAIR] column, used only for pacing
COST = {"dve_h": 1.25, "dve_h2": 3.4, "act": 3.75, "pool": 5.8}
N_POOL_COLS = 0             # gpsimd fp16 tensor ops measured ~10x slower than
                            # the cost model on HW -- keep the Pool engine off
                            # the column path
SIM_ERR_ACCEPT = 9e-3       # accept a spec if simulated rel err is below


# ----------------------------------------------------------------------------
# host-side: radial function, kink enumeration, greedy basis fit
def _radial_fn(r, W1, b1, W2, b2):
    radii = np.linspace(0.0, MAX_RADIUS, N_BASIS)
    step = radii[1] - radii[0]
    x = (r[..., None] - radii) / step
    basis = np.where(np.abs(x) < 1.0, np.cos(0.5 * np.pi * x) ** 2, 0.0)
    hid = np.maximum(basis @ W1 + b1, 0.0)
    return (hid @ W2 + b2) * Y00


def _find_kinks(W1, W2):
    """Analytic relu-kink locations of K(r) in phi-space, sorted by |jump|."""
    out = []
    regions = [
        (W1[1], W1[0] - W1[1]),
        (W1[1], W1[2] - W1[1]),
        (np.zeros(W1.shape[1]), W1[2]),
    ]
    for R, (c, d) in enumerate(regions):
        for h in range(W1.shape[1]):
            if abs(d[h]) < 1e-12:
                continue
            u = -c[h] / d[h]
            if not (0.0 < u < 1.0):
                continue
            t = np.arccos(np.clip(2 * u - 1, -1, 1))
            theta = R * np.pi + (t if R % 2 == 0 else np.pi - t)
            rstar = theta * 1.5 / np.pi
            dudr = -np.pi / 3.0 * np.sin(np.pi * rstar / 1.5)
            jump = abs(d[h] * dudr) * np.linalg.norm(W2[h]) * Y00
            out.append((np.pi * rstar / RCUT, jump))
    out.sort(key=lambda x: -x[1])
    return out


def _basis_columns(phi, spec):
    cols = []
    for item in spec:
        kind = item[0]
        if kind == "const":
            cols.append(np.ones_like(phi))
        elif kind == "lin":
            cols.append(phi)
        elif kind == "tanh":
            _, s, c = item
            cols.append(np.tanh(s * phi - s * c))
        elif kind == "h":
            cols.append(np.maximum(phi - item[1], 0.0))
        elif kind == "h2":
            cols.append(np.maximum(phi - item[1], 0.0) ** 2)
        else:
            raise ValueError(kind)
    return np.stack(cols, -1)


def _fit_grid(W1, b1, W2, b2):
    npts = 8192
    phig = (np.arange(npts) + 0.5) / npts * np.pi
    # clamped pairs (r >= 4.5) land exactly at phi=pi; the diagonal at 0.
    phig = np.concatenate([phig, np.full(96, np.pi), np.zeros(16)])
    Fg = _radial_fn(phig * RCUT / np.pi, W1, b1, W2, b2)
    return phig, Fg


def _fit_alpha(spec, phig, Fg, lam=1e-3):
    A = _basis_columns(phig, spec)
    lamr = lam * math.sqrt(A.shape[0])
    Aaug = np.concatenate([A, lamr * np.eye(len(spec))], 0)
    Faug = np.concatenate([Fg, np.zeros((len(spec), Fg.shape[1]))], 0)
    alpha, *_ = np.linalg.lstsq(Aaug, Faug, rcond=None)
    return alpha


def _greedy_spec(W1, b1, W2, b2, n_total, max_tanh, lam=1e-3, with_h2=False):
    """Greedy forward selection (OMP-style) of basis columns."""
    kinks = _find_kinks(W1, W2)
    phig, Fg = _fit_grid(W1, b1, W2, b2)
    cands = [("tanh", float(s), float(c)) for s, c in CAND_TANH]
    cands += [("h", float(p)) for p, _ in kinks]
    cands += [("h", float(p)) for p in (np.arange(96) + 0.5) / 96 * np.pi]
    if with_h2:
        cands += [("h2", float(p)) for p in (np.arange(48) + 0.5) / 48 * np.pi]
    A_all = _basis_columns(phig, cands)
    colnorm = np.linalg.norm(A_all, axis=0) + 1e-30

    sel = set()
    spec = [("const",), ("lin",)]
    while len(sel) < n_total:
        alpha = _fit_alpha(spec, phig, Fg, lam)
        res = Fg - _basis_columns(phig, spec) @ alpha
        gain = ((A_all.T @ res) ** 2).sum(1) / colnorm ** 2
        ntanh = sum(1 for s in spec if s[0] == "tanh")
        for j in np.argsort(-gain):
            j = int(j)
            if j in sel:
                continue
            if cands[j][0] == "tanh" and ntanh >= max_tanh:
                continue
            sel.add(j)
            spec.append(cands[j])
            break
        else:
            break
    alpha = _fit_alpha(spec, phig, Fg, lam)
    return spec, alpha


# ----------------------------------------------------------------------------
# host-side: full-pipeline fp16 error simulation against an interp reference
def _sim_rel_err(spec, alpha, inputs_np):
    f = inputs_np["features"][:, :, 0, :]
    geo = inputs_np["geometry"][:, :, 0, :]
    W1, b1 = inputs_np["W1"].astype(np.float64), inputs_np["b1"].astype(np.float64)
    W2, b2 = inputs_np["W2"].astype(np.float64), inputs_np["b2"].astype(np.float64)

    # reference K on a dense uniform phi grid for linear interpolation
    ngrid = 65536
    pg = np.arange(ngrid + 1) / ngrid * np.pi
    Kg = _radial_fn(pg * RCUT / np.pi, W1, b1, W2, b2).astype(np.float32)

    alphaf = (alpha.T / math.sqrt(N)).astype(np.float32)        # [C, Q]
    conv_ref = np.zeros((B, N), np.float32)
    conv_dev = np.zeros((B, N), np.float32)
    for z in range(B):
        x = geo[z]
        d = x[:, None, :] - x[None, :, :]
        r2 = (d * d).sum(-1)
        phi = np.sqrt(np.clip(r2, 1e-12, RCUT * RCUT)) * (np.pi / RCUT)
        # reference: interp K rows, contract with f
        pos = phi * (ngrid / np.pi)
        idx = np.minimum(pos.astype(np.int64), ngrid - 1)
        frac = (pos - idx).astype(np.float32)[..., None]
        Kv = Kg[idx] * (1 - frac) + Kg[idx + 1] * frac           # [b,a,C]
        conv_ref[z] = np.einsum('baC,bC->a', Kv, f[z]) / math.sqrt(N)
        # device: fp16 everywhere
        phi16 = phi.astype(np.float16).astype(np.float32)
        gq = (f[z] @ alphaf).astype(np.float16).astype(np.float32)  # [b, Q]
        acc = np.zeros(N, np.float32)
        for qi, item in enumerate(spec):
            T = _basis_columns(phi16, [item])[..., 0].astype(np.float16)
            acc += gq[:, qi] @ T.astype(np.float32)
        conv_dev[z] = acc

    def head(conv):
        h1 = np.maximum(conv @ inputs_np["Wfc1"] + inputs_np["bfc1"], 0)
        h2 = np.maximum(h1 @ inputs_np["Wfc2"] + inputs_np["bfc2"], 0)
        h3 = np.maximum(h2 @ inputs_np["Wfc3"] + inputs_np["bfc3"], 0)
        return h3[:, 0]

    out_ref, out_dev = head(conv_ref), head(conv_dev)
    scale = max(np.abs(out_ref).max(), 1e-9)
    return float(np.abs(out_ref - out_dev).max() / scale)


# ----------------------------------------------------------------------------
# emission plan: engine assignment + finish-time pacing
def _plan_emission(spec):
    """Returns ordered list of (qi, item, engine) defining device emission.

    engine in {"free", "dve", "act", "pool"}; "free" columns (const/lin)
    have no producer op.
    """
    free, act_q, dve_q, pool_q = [], [], [], []
    nh = 0
    n_h_total = sum(1 for it in spec if it[0] == "h")
    pool_stride = max(2, (n_h_total + N_POOL_COLS - 1) // max(N_POOL_COLS, 1))
    n_pool = 0
    for qi, item in enumerate(spec):
        kind = item[0]
        if kind in ("const", "lin"):
            free.append((qi, item, "free"))
        elif kind == "tanh":
            act_q.append((qi, item, "act"))
        elif kind == "h":
            nh += 1
            if n_pool < N_POOL_COLS and nh % pool_stride == 0:
                pool_q.append((qi, item, "pool"))
                n_pool += 1
            else:
                dve_q.append((qi, item, "dve"))
        elif kind == "h2":
            dve_q.append((qi, item, "dve"))
        else:
            raise ValueError(kind)

    # merge queues by simulated finish time
    t_eng = {"act": 0.0, "dve": 0.0, "pool": 0.0}
    qs = {"act": act_q, "dve": dve_q, "pool": pool_q}
    order = list(free)
    while any(qs.values()):
        best, best_t = None, None
        for eng, q in qs.items():
            if not q:
                continue
            item = q[0]
            c = COST["act"] if eng == "act" else (
                COST["pool"] if eng == "pool" else
                (COST["dve_h2"] if item[1][0] == "h2" else COST["dve_h"]))
            ft = t_eng[eng] + c
            if best_t is None or ft < best_t:
                best, best_t = eng, ft
        item = qs[best].pop(0)
        t_eng[best] = best_t
        order.append(item)
    return order


# ----------------------------------------------------------------------------
# device program
def _layout_key(order):
    """Program structure key: kinds+engines in emission order (params are
    baked in as immediates, so they are part of the key too)."""
    return tuple((qi, item, eng) for qi, item, eng in order)


def _const_layout(Q, n_tanh):
    """Column offsets inside the packed consts blob [128, W] fp32."""
    off = {}
    w = 0
    for name, cols in [("alphaT", Q), ("wfc1p", 120), ("bfc1", 1),
                       ("wfc2", 10), ("bfc2", 1), ("wfc3", 1), ("bfc3", 1),
                       ("actbias", max(n_tanh, 1))]:
        off[name] = (w, w + cols)
        w += cols
    return off, w


def _build_program(order, use_pool=True):
    Q = len(order)
    n_tanh = sum(1 for _, it, _ in order if it[0] == "tanh")
    coff, cw = _const_layout(Q, n_tanh)
    nc = bacc.Bacc("TRN2", target_bir_lowering=False, debug=False)

    AB_d = nc.dram_tensor("AB", [5, 2 * BPER * N], F32R, kind="ExternalInput").ap()
    fT_d = nc.dram_tensor("fT", [C, BPER * N], F32, kind="ExternalInput").ap()
    consts_d = nc.dram_tensor("consts", [128, cw], F32, kind="ExternalInput").ap()
    out_d = nc.dram_tensor("out", [1, BPER], F32, kind="ExternalOutput").ap()

    with tile.TileContext(nc) as tc, ExitStack() as ctx:
        sb = ctx.enter_context(tc.tile_pool(name="sb", bufs=1))
        pconv = ctx.enter_context(tc.tile_pool(name="pconv", space="PSUM", bufs=1))
        p_g = ctx.enter_context(tc.tile_pool(name="p_g", space="PSUM", bufs=2))
        p_r2 = ctx.enter_context(tc.tile_pool(name="p_r2", space="PSUM", bufs=2))
        p_w = ctx.enter_context(tc.tile_pool(name="p_w", space="PSUM", bufs=1))
        p_fc = ctx.enter_context(tc.tile_pool(name="p_fc", space="PSUM", bufs=1))
        cpool = ctx.enter_context(tc.tile_pool(name="cpool", bufs=3))
        tpool = ctx.enter_context(tc.tile_pool(name="tpool", bufs=8))

        # ---- inputs to SBUF: one big DMA per ring (each InstDMACopy is
        # split across all 16 SDMA engines internally; per-dma fixed cost
        # ~2us, so fewer+bigger wins).  AB (gates the phi critical path)
        # goes on the ACT hwdge ring, the rest on the SP ring.
        AB = sb.tile([5, 2 * BPER * N], F32R, name="AB_sb")
        fT = sb.tile([C, BPER * N], F32, name="fT_sb")
        consts = sb.tile([128, cw], F32, name="consts_sb")
        phi = sb.tile([128, NPAIR], F16, name="phi")
        nc.scalar.dma_start(out=AB, in_=AB_d)
        nc.sync.dma_start(out=consts, in_=consts_d)
        nc.sync.dma_start(out=fT, in_=fT_d)
        lhsA, rhsB = AB[:, 0:BPER * N], AB[:, BPER * N:2 * BPER * N]

        def cs(name, rows):
            a, b = coff[name]
            return consts[0:rows, a:b]
        alphaT = cs("alphaT", C)
        wfc1p = cs("wfc1p", 128)
        bfc1 = cs("bfc1", 30)
        wfc2 = cs("wfc2", 30)
        bfc2 = cs("bfc2", 10)
        wfc3 = cs("wfc3", 10)
        bfc3 = cs("bfc3", 1)
        actbias = cs("actbias", 128)

        # ---- working tiles
        ones = sb.tile([128, N], F16, name="ones")
        gT = sb.tile([128, BPER * 4 * Q], F16, name="gT")
        convrow = sb.tile([1, BPER * N], F32, name="convrow")
        ones1 = sb.tile([1, 1], F32, name="ones1")
        convcol = sb.tile([128, BPER * 4], F32, name="convcol")
        h1 = sb.tile([30, BPER], F32, name="h1")
        h2 = sb.tile([10, BPER], F32, name="h2")
        out_sb = sb.tile([1, BPER], F32, name="out_sb")

        psum_conv = [pconv.tile([128, N], F32, name=f"pconv{z}", tag=f"pconv{z}")
                     for z in range(BPER)]

        nc.vector.memset(ones, 1.0)
        nc.vector.memset(ones1, 1.0)

        # ---- PE warm-up: dummy matmuls during the input DMAs push the
        # HAM p-state toward full clock before the real work arrives
        warm_ps = p_w.tile([1, N], F32, name="warm", tag="p_w")
        for _ in range(6):
            nc.tensor.matmul(warm_ps, ones[:, 0:1], ones,
                             start=True, stop=True,
                             tile_position=(0, 0), skip_group_check=True)

        # ---- pairwise r^2 -> phi = min(sqrt(max(r2,1e-12)) * pi/4.5, pi)
        for z in range(BPER):
            for bc in range(4):
                pr2 = p_r2.tile([128, N], F32, name="pr2", tag="p_r2")
                nc.tensor.matmul(
                    pr2,
                    lhsA[:, z * N + bc * 128: z * N + (bc + 1) * 128],
                    rhsB[:, z * N:(z + 1) * N],
                )
                cl = cpool.tile([128, N], F32, name="cl", tag="clamp")
                nc.vector.tensor_scalar(cl, pr2, 1e-12, RCUT * RCUT,
                                        ALU.max, ALU.min)
                nc.scalar.activation(
                    phi[:, (z * 4 + bc) * N:(z * 4 + bc + 1) * N],
                    cl, AF.Sqrt, bias=0.0, scale=(math.pi / RCUT) ** 2)

        # ---- g[q, b] = sum_c alpha[q,c] f[b,c] / sqrt(N), laid out [b, q] f16
        for z in range(BPER):
            for bc in range(4):
                pg = p_g.tile([128, Q], F32, name="pg", tag="p_g")
                nc.tensor.matmul(
                    pg,
                    fT[:, z * N + bc * 128: z * N + (bc + 1) * 128],
                    alphaT,
                )
                o = (z * 4 + bc) * Q
                nc.vector.tensor_copy(gT[:, o:o + Q], pg)

        # ---- main loop: batches of 4 columns; 4-way col-tiled accumulation
        eng_map = {"dve": nc.vector, "pool": nc.gpsimd if use_pool else nc.vector}
        n_lanes = min(4, len(order))
        lane_count = [0] * n_lanes
        for i in range(len(order)):
            lane_count[i % n_lanes] += 1
        lane_seen = [0] * n_lanes
        bias_i = 0

        for b0 in range(0, len(order), n_lanes):
            batch = order[b0:b0 + n_lanes]
            rhs_tiles = []
            for qi, item, eng in batch:
                kind = item[0]
                if kind == "const":
                    rhs_tiles.append(ones)
                elif kind == "lin":
                    rhs_tiles.append(phi)
                elif kind == "tanh":
                    t_t = tpool.tile([128, NPAIR], F16, name="t_t", tag="T")
                    nc.scalar.activation(t_t, phi, AF.Tanh,
                                         bias=actbias[:, bias_i:bias_i + 1],
                                         scale=float(item[1]))
                    bias_i += 1
                    rhs_tiles.append(t_t)
                elif kind == "h":
                    t_t = tpool.tile([128, NPAIR], F16, name="t_t", tag="T")
                    eng_map[eng].tensor_scalar(t_t, phi, float(item[1]), 0.0,
                                               ALU.subtract, ALU.max)
                    rhs_tiles.append(t_t)
                elif kind == "h2":
                    t_t = tpool.tile([128, NPAIR], F16, name="t_t", tag="T")
                    nc.vector.tensor_scalar(t_t, phi, float(item[1]), 0.0,
                                            ALU.subtract, ALU.max)
                    nc.vector.tensor_tensor(t_t, t_t, t_t, ALU.mult)
                    rhs_tiles.append(t_t)
                else:
                    raise ValueError(kind)

            for lane in range(len(batch)):
                lane_seen[lane] += 1
            for z in range(BPER):
                for bc in range(4):
                    for lane, (qi, item, eng) in enumerate(batch):
                        row = 32 * lane
                        col = (z * 4 + bc) * len(order) + qi
                        rt = rhs_tiles[lane]
                        rhs = (rt if rt is ones else
                               rt[:, (z * 4 + bc) * N:(z * 4 + bc + 1) * N])
                        nc.tensor.matmul(
                            psum_conv[z][row:row + 1, :],
                            gT[:, col:col + 1],
                            rhs,
                            start=(lane_seen[lane] == 1 and bc == 0),
                            stop=(lane_seen[lane] == lane_count[lane] and bc == 3),
                            tile_position=(0, row),
                            skip_group_check=True,
                        )

        # ---- combine the 4 partial rows (engine writes must be 32-aligned,
        # so accumulate into one partition-0 row), then K=1 matmuls
        # transpose each 128-block into convcol -- no DRAM bounce
        for z in range(BPER):
            ps = psum_conv[z]
            sl = slice(z * N, (z + 1) * N)
            nc.vector.tensor_copy(convrow[0:1, sl], ps[0:1, :])
            for row in (32, 64, 96):
                nc.vector.tensor_tensor(convrow[0:1, sl], ps[row:row + 1, :],
                                        convrow[0:1, sl], ALU.add)
        for z in range(BPER):
            pcc = p_fc.tile([128, 4], F32, name="pcc", tag="p_fc")
            for j in range(4):
                nc.tensor.matmul(
                    pcc[:, j:j + 1],
                    convrow[0:1, z * N + j * 128: z * N + (j + 1) * 128],
                    ones1,
                    start=True, stop=True,
                )
            nc.vector.tensor_copy(convcol[:, z * 4:(z + 1) * 4], pcc)
            pfc1 = p_fc.tile([30, 1], F32, name="pfc1", tag="p_fc")
            for j in range(4):
                nc.tensor.matmul(
                    pfc1,
                    wfc1p[:, j * 30:(j + 1) * 30],
                    convcol[:, z * 4 + j: z * 4 + j + 1],
                    start=(j == 0), stop=(j == 3),
                )
            nc.scalar.activation(h1[:, z:z + 1], pfc1, AF.Relu, bias=bfc1, scale=1.0)
            pfc2 = p_fc.tile([10, 1], F32, name="pfc2", tag="p_fc")
            nc.tensor.matmul(pfc2, wfc2, h1[:, z:z + 1])
            nc.scalar.activation(h2[:, z:z + 1], pfc2, AF.Relu, bias=bfc2, scale=1.0)
            pfc3 = p_fc.tile([1, 1], F32, name="pfc3", tag="p_fc")
            nc.tensor.matmul(pfc3, wfc3, h2[:, z:z + 1])
            nc.scalar.activation(out_sb[0:1, z:z + 1], pfc3, AF.Relu, bias=bfc3,
                                 scale=1.0)

        nc.sync.dma_start(out=out_d, in_=out_sb)

    nc.compile()
    return nc


# ----------------------------------------------------------------------------
_CACHE = {}
LAST_RESULT = None


def kernel(features, geometry, W1, b1, W2, b2,
           Wfc1, bfc1, Wfc2, bfc2, Wfc3, bfc3):
    global LAST_RESULT
    inputs_np = {
        "features": np.asarray(features, np.float32),
        "geometry": np.asarray(geometry, np.float32),
        "W1": np.asarray(W1, np.float32), "b1": np.asarray(b1, np.float32),
        "W2": np.asarray(W2, np.float32), "b2": np.asarray(b2, np.float32),
        "Wfc1": np.asarray(Wfc1, np.float32), "bfc1": np.asarray(bfc1, np.float32),
        "Wfc2": np.asarray(Wfc2, np.float32), "bfc2": np.asarray(bfc2, np.float32),
        "Wfc3": np.asarray(Wfc3, np.float32), "bfc3": np.asarray(bfc3, np.float32),
    }
    W1d = inputs_np["W1"].astype(np.float64)
    b1d = inputs_np["b1"].astype(np.float64)
    W2d = inputs_np["W2"].astype(np.float64)
    b2d = inputs_np["b2"].astype(np.float64)

    # choose a spec: fastest first, fall back if simulated error too high
    configs = [
        dict(n_total=38, max_tanh=10, lam=1e-3, with_h2=False),
        dict(n_total=40, max_tanh=8, lam=1e-3, with_h2=True),
        dict(n_total=46, max_tanh=12, lam=1e-3, with_h2=True),
    ]
    best = None
    for cfg in configs:
        spec, alpha = _greedy_spec(W1d, b1d, W2d, b2d, **cfg)
        err = _sim_rel_err(spec, alpha, inputs_np)
        if best is None or err < best[2]:
            best = (spec, alpha, err)
        if err <= SIM_ERR_ACCEPT:
            best = (spec, alpha, err)
            break
    spec, alpha, sim_err = best
    if os.environ.get("KERNEL_VERBOSE"):
        nt = sum(1 for s in spec if s[0] == "tanh")
        nh = sum(1 for s in spec if s[0] == "h")
        nh2 = sum(1 for s in spec if s[0] == "h2")
        print(f"[kernel] spec Q={len(spec)} tanh={nt} h={nh} h2={nh2} "
              f"sim_err={sim_err:.3e}")

    order = _plan_emission(spec)
    key = _layout_key(order)
    if key not in _CACHE:
        _CACHE[key] = _build_program(order)
    nc = _CACHE[key]

    Q = len(spec)
    n_tanh = sum(1 for s in spec if s[0] == "tanh")
    coff, cw = _const_layout(Q, n_tanh)
    bias_vals = np.array(
        [-item[1] * item[2] for _, item, _ in order if item[0] == "tanh"],
        dtype=np.float32)
    blob = np.zeros((128, cw), np.float32)

    def put(name, arr):
        a, b = coff[name]
        arr = np.asarray(arr, np.float32)
        blob[:arr.shape[0], a:a + arr.shape[1]] = arr
    put("alphaT", (alpha.T / math.sqrt(N)))                   # [C, Q]
    put("wfc1p", inputs_np["Wfc1"].reshape(4, 128, 30)
        .transpose(1, 0, 2).reshape(128, 120))
    put("bfc1", inputs_np["bfc1"].reshape(30, 1))
    put("wfc2", inputs_np["Wfc2"])
    put("bfc2", inputs_np["bfc2"].reshape(10, 1))
    put("wfc3", inputs_np["Wfc3"])
    put("bfc3", inputs_np["bfc3"].reshape(1, 1))
    if bias_vals.size:
        put("actbias", np.broadcast_to(bias_vals[None, :],
                                       (128, bias_vals.size)))
    consts = {"consts": blob}
    in_maps = []
    for core in range(NCORES):
        zs = slice(core * BPER, (core + 1) * BPER)
        geoT = inputs_np["geometry"][zs, :, 0, :].transpose(2, 0, 1).reshape(3, BPER * N)
        nsq = (geoT * geoT).sum(0, keepdims=True)        # [1, BPER*N]
        onesv = np.ones_like(nsq)
        lhsA = np.concatenate([onesv, nsq, -2.0 * geoT], 0)
        rhsB = np.concatenate([nsq, onesv, geoT], 0)
        AB = np.ascontiguousarray(
            np.concatenate([lhsA, rhsB], 1).astype(np.float32))
        fTm = np.ascontiguousarray(
            inputs_np["features"][zs, :, 0, :].transpose(2, 0, 1).reshape(C, BPER * N))
        in_maps.append({"AB": AB, "fT": fTm, **consts})

    from concourse.bass_utils import run_bass_kernel_spmd
    trace = bool(int(os.environ.get("KERNEL_TRACE", "0")))
    res = run_bass_kernel_spmd(nc, in_maps, list(range(NCORES)), trace=trace)
    LAST_RESULT = res

    out = np.concatenate([res.results[c]["out"].reshape(BPER)
                          for c in range(NCORES)])
    return out.astype(np.float32)
